# revision 22
# baseline (speedup 1.0000x reference)
"""Trainium2 Bass kernel for the DecoderTP temporal-Hawkes decoder.

Full-input contract: kernel(**inputs) takes the unsharded numpy inputs and
returns the full output tuple (loss_lambda/E, loss_surv/E, cond_pos, cond_neg).

Sharding: data-parallel over the event dim E (8 cores x 1024 events).
 - u/v survival embeddings are reordered host-side to per-core contiguous
   [1024, S*2*D] blocks (pure data movement).
 - all_embeddings and last_update are merged into one [N, D+1] table,
   replicated to every core; rows are gathered on-device via indirect DMA.
 - The two scalar losses are produced as per-core partials (already scaled
   by -1/E resp. 1/E) and summed host-side (the all-reduce).

Math used on device (exact algebraic rewrite of the reference):
 g_uv + g_vu = (zu+zv) @ (Wu+Wv).T  ==>  per-row fused multiply-reduce
 against per-event selected weights w_sel = wsum0 + et*(wsum1-wsum0).
"""

import numpy as np

from concourse import bass, mybir, tile
from concourse import bacc
from concourse import bass_utils

# Problem constants (hardcoded per harness contract).
E = 8192
D = 256
S = 20
N = 100000
EPS = 1e-7
NCORES = 8
EC = E // NCORES          # events per core = 1024
P = 128                   # partitions
T = EC // P               # event tiles per core = 8
ROW = D + 1               # merged table row: embedding + last_update
F32 = mybir.dt.float32
I32 = mybir.dt.int32

_CACHED_NC = None


def _build_nc(debug=False):
    # Bacc (not raw Bass): its finalize() runs generate_event_semaphores(),
    # which legalizes multi-semaphore waits (HW allows one wait/instruction),
    # and codegen_inst_isa_subclasses() for the fused TTR instructions.
    nc = bacc.Bacc()

    uv = nc.declare_dram_parameter("uv", [EC, S * 2 * D], F32, isOutput=False)
    tab = nc.declare_dram_parameter("tab", [N, ROW], F32, isOutput=False)
    idx = nc.declare_dram_parameter("idx", [EC, 3], I32, isOutput=False)
    etd = nc.declare_dram_parameter("etd", [EC, 1], I32, isOutput=False)
    ctd = nc.declare_dram_parameter("ctd", [EC, 1], F32, isOutput=False)
    wb = nc.declare_dram_parameter("wb", [P, 4 * D], F32, isOutput=False)
    pars = nc.declare_dram_parameter("pars", [P, 8 + S], F32, isOutput=False)
    out_cond = nc.declare_dram_parameter("out_cond", [P, 2 * T], F32, isOutput=True)
    out_loss = nc.declare_dram_parameter("out_loss", [1, 2], F32, isOutput=True)
    if debug:
        out_dbg = nc.declare_dram_parameter("out_dbg", [P, 64], F32, isOutput=True)

    with tile.TileContext(nc) as tc:
        with (
            tc.tile_pool(name="big", bufs=2) as big,
            tc.tile_pool(name="gath", bufs=2) as gathp,
            tc.tile_pool(name="work", bufs=2) as work,
            tc.tile_pool(name="small", bufs=3) as small,
            tc.tile_pool(name="persist", bufs=1) as persist,
            tc.tile_pool(name="psum", bufs=1, space="PSUM") as psump,
        ):
            # ---- one-time prologue -------------------------------------
            wtile = persist.tile([P, 4 * D], F32, tag="wtile")
            nc.sync.dma_start(out=wtile[:], in_=wb[:])
            # wsum[:, 0:D] = Wu0+Wv0 ; wsum[:, D:2D] = Wu1+Wv1 (bcast rows)
            wsum = persist.tile([P, 2 * D], F32, tag="wsum")
            nc.vector.tensor_tensor(
                out=wsum[:, 0:D], in0=wtile[:, 0:D], in1=wtile[:, D : 2 * D],
                op=mybir.AluOpType.add,
            )
            nc.vector.tensor_tensor(
                out=wsum[:, D : 2 * D], in0=wtile[:, 2 * D : 3 * D],
                in1=wtile[:, 3 * D : 4 * D], op=mybir.AluOpType.add,
            )
            # fold the 0.5 of G = 0.5*(zu+zv)@(Wu+Wv).T into the weights
            nc.vector.tensor_scalar(
                out=wsum[:], in0=wsum[:], scalar1=0.5, scalar2=None,
                op0=mybir.AluOpType.mult)
            wdiff = persist.tile([P, D], F32, tag="wdiff")
            nc.vector.tensor_tensor(
                out=wdiff[:], in0=wsum[:, D : 2 * D], in1=wsum[:, 0:D],
                op=mybir.AluOpType.subtract,
            )

            parst = persist.tile([P, 8 + S], F32, tag="parst")
            nc.sync.dma_start(out=parst[:], in_=pars[:])
            # pdiff cols: 0 = b1-b0, 1 = psi1-psi0, 2 = alpha1-alpha0
            pdiff = persist.tile([P, 3], F32, tag="pdiff")
            for k in range(3):
                nc.vector.tensor_tensor(
                    out=pdiff[:, k : k + 1],
                    in0=parst[:, 2 * k + 1 : 2 * k + 2],
                    in1=parst[:, 2 * k : 2 * k + 1],
                    op=mybir.AluOpType.subtract,
                )
            p0e = persist.tile([P, 1], F32, tag="p0e")  # psi0 + EPS
            nc.vector.tensor_scalar(
                out=p0e[:], in0=parst[:, 2:3], scalar1=EPS, scalar2=None,
                op0=mybir.AluOpType.add,
            )

            iota_f = parst[:, 8 : 8 + S]

            ones = persist.tile([P, 1], F32, tag="ones")
            nc.vector.memset(ones[:], 1.0)
            epsc = persist.tile([P, 1], F32, tag="epsc")
            nc.vector.memset(epsc[:], EPS)

            # per-event accumulators (col t = event tile t)
            loga = persist.tile([P, T], F32, tag="loga")
            intega = persist.tile([P, T], F32, tag="intega")
            condp = persist.tile([P, T], F32, tag="condp")
            condn = persist.tile([P, T], F32, tag="condn")

            ADD = mybir.AluOpType.add
            SUB = mybir.AluOpType.subtract
            MUL = mybir.AluOpType.mult
            MAX = mybir.AluOpType.max
            MIN = mybir.AluOpType.min

            # ---- main loop over event tiles ----------------------------
            for t in range(T):
                r0 = t * P
                uvt = big.tile([P, S * 2 * D], F32, tag="uvt")
                nc.sync.dma_start(out=uvt[:], in_=uv[r0 : r0 + P, :])

                idxt = small.tile([P, 3], I32, tag="idxt")
                nc.sync.dma_start(out=idxt[:], in_=idx[r0 : r0 + P, :])
                ett = small.tile([P, 1], I32, tag="ett")
                nc.sync.dma_start(out=ett[:], in_=etd[r0 : r0 + P, :])
                ctt = small.tile([P, 1], F32, tag="ctt")
                nc.sync.dma_start(out=ctt[:], in_=ctd[r0 : r0 + P, :])

                gth = gathp.tile([P, 3 * ROW], F32, tag="gth")
                for j in range(3):
                    nc.gpsimd.indirect_dma_start(
                        out=gth[:, j * ROW : (j + 1) * ROW],
                        out_offset=None,
                        in_=tab[:],
                        in_offset=bass.IndirectOffsetOnAxis(
                            ap=idxt[:, j : j + 1], axis=0
                        ),
                    )

                etf = small.tile([P, 1], F32, tag="etf")
                nc.vector.tensor_copy(out=etf[:], in_=ett[:])

                # per-event selected params
                b_sel = small.tile([P, 1], F32, tag="b_sel")
                nc.vector.tensor_scalar(
                    out=b_sel[:], in0=etf[:], scalar1=pdiff[:, 0:1],
                    scalar2=parst[:, 0:1], op0=MUL, op1=ADD)
                psi_sel = small.tile([P, 1], F32, tag="psi_sel")
                nc.vector.tensor_scalar(
                    out=psi_sel[:], in0=etf[:], scalar1=pdiff[:, 1:2],
                    scalar2=parst[:, 2:3], op0=MUL, op1=ADD)
                psi_eps = small.tile([P, 1], F32, tag="psi_eps")
                nc.vector.tensor_scalar(
                    out=psi_eps[:], in0=etf[:], scalar1=pdiff[:, 1:2],
                    scalar2=p0e[:], op0=MUL, op1=ADD)
                inv_psi = small.tile([P, 1], F32, tag="inv_psi")
                nc.vector.reciprocal(out=inv_psi[:], in_=psi_eps[:])
                alpha_sel = small.tile([P, 1], F32, tag="alpha_sel")
                nc.vector.tensor_scalar(
                    out=alpha_sel[:], in0=etf[:], scalar1=pdiff[:, 2:3],
                    scalar2=parst[:, 4:5], op0=MUL, op1=ADD)

                # selected weight row: wsum0 + et*(wsum1-wsum0), duplicated
                # into both halves of sel2 so uv-step rows multiply directly
                sel2 = work.tile([P, 2 * D], F32, tag="sel2")
                nc.vector.tensor_scalar(
                    out=sel2[:, 0:D], in0=wdiff[:], scalar1=etf[:],
                    scalar2=None, op0=MUL)
                nc.vector.tensor_tensor(
                    out=sel2[:, 0:D], in0=sel2[:, 0:D], in1=wsum[:, 0:D],
                    op=ADD)
                nc.vector.tensor_copy(out=sel2[:, D : 2 * D], in_=sel2[:, 0:D])
                sel = sel2[:, 0:D]

                lu_src = gth[:, D : D + 1]
                lu_pos = gth[:, ROW + D : ROW + D + 1]
                lu_neg = gth[:, 2 * ROW + D : 2 * ROW + D + 1]

                # Temporals first: these standard DVE ops are the first
                # consumers of the gathered tile + the uv stream, so they
                # absorb the DMA semaphore waits (the fused TTR ISA insts
                # below have very few sync-wait slots).
                ltp = small.tile([P, 1], F32, tag="ltp")
                nc.vector.tensor_tensor(out=ltp[:], in0=lu_src, in1=lu_pos, op=MAX)
                ltn = small.tile([P, 1], F32, tag="ltn")
                nc.vector.tensor_tensor(out=ltn[:], in0=lu_src, in1=lu_neg, op=MAX)
                touch = small.tile([P, 1], F32, tag="touch")
                nc.vector.tensor_copy(out=touch[:], in_=uvt[:, 0:1])

                td = small.tile([P, 1], F32, tag="td")
                nc.vector.tensor_tensor(out=td[:], in0=ctt[:], in1=ltp[:], op=SUB)
                lt1 = small.tile([P, 1], F32, tag="lt1")
                nc.vector.tensor_scalar(
                    out=lt1[:], in0=ltp[:], scalar1=1.0, scalar2=None, op0=ADD)
                rec = small.tile([P, 1], F32, tag="rec")
                nc.vector.reciprocal(out=rec[:], in_=lt1[:])
                tpos = small.tile([P, 1], F32, tag="tpos")
                nc.vector.tensor_tensor(out=tpos[:], in0=td[:], in1=rec[:], op=MUL)

                # dot products for pos/neg pairs: 0.5 factor is in sel.
                # z_src*sel is shared between the pos and neg dots.
                scr = work.tile([P, 2 * D], F32, tag="scr")
                scr3 = scr[:].rearrange("p (a b) -> p a b", a=2)
                nc.vector.tensor_tensor(
                    out=scr[:, 0:D], in0=gth[:, 0:D], in1=sel, op=MUL)
                nc.vector.tensor_tensor(
                    out=scr[:, D : 2 * D], in0=gth[:, ROW : ROW + D],
                    in1=sel, op=MUL)
                d_pos = small.tile([P, 1], F32, tag="d_pos")
                nc.vector.tensor_reduce(
                    out=d_pos[:], in_=scr3, axis=mybir.AxisListType.XY,
                    op=ADD)
                nc.vector.tensor_tensor(
                    out=scr[:, D : 2 * D], in0=gth[:, 2 * ROW : 2 * ROW + D],
                    in1=sel, op=MUL)
                d_neg = small.tile([P, 1], F32, tag="d_neg")
                nc.vector.tensor_reduce(
                    out=d_neg[:], in_=scr3, axis=mybir.AxisListType.XY,
                    op=ADD)

                def intensity(dot_ap, temporal_ap, tag):
                    # psi * softplus(clip((dot + b + alpha*temporal)/psi_eps))
                    g = small.tile([P, 1], F32, tag=tag + "_g")
                    nc.vector.tensor_scalar(
                        out=g[:], in0=temporal_ap, scalar1=alpha_sel[:],
                        scalar2=b_sel[:], op0=MUL, op1=ADD)
                    nc.vector.tensor_tensor(out=g[:], in0=g[:], in1=dot_ap, op=ADD)
                    nc.vector.tensor_scalar(
                        out=g[:], in0=g[:], scalar1=inv_psi[:], scalar2=-75.0,
                        op0=MUL, op1=MAX)
                    nc.vector.tensor_scalar(
                        out=g[:], in0=g[:], scalar1=75.0, scalar2=None, op0=MIN)
                    # stable softplus: relu(g) + ln(1 + exp(-|g|))
                    ga = small.tile([P, 1], F32, tag=tag + "_ga")
                    nc.scalar.activation(
                        out=ga[:], in_=g[:],
                        func=mybir.ActivationFunctionType.Abs)
                    ex = small.tile([P, 1], F32, tag=tag + "_ex")
                    nc.scalar.activation(
                        out=ex[:], in_=ga[:],
                        func=mybir.ActivationFunctionType.Exp, scale=-1.0)
                    ln1 = small.tile([P, 1], F32, tag=tag + "_ln1")
                    nc.scalar.activation(
                        out=ln1[:], in_=ex[:],
                        func=mybir.ActivationFunctionType.Ln, bias=1.0)
                    rl = small.tile([P, 1], F32, tag=tag + "_rl")
                    nc.scalar.activation(
                        out=rl[:], in_=g[:],
                        func=mybir.ActivationFunctionType.Relu)
                    sp = small.tile([P, 1], F32, tag=tag + "_sp")
                    nc.vector.tensor_tensor(
                        out=sp[:], in0=rl[:], in1=ln1[:], op=ADD)
                    lam = small.tile([P, 1], F32, tag=tag + "_lam")
                    nc.vector.tensor_scalar(
                        out=lam[:], in0=sp[:], scalar1=psi_sel[:],
                        scalar2=None, op0=MUL)
                    return lam

                lam_pos = intensity(d_pos[:], tpos[:], "pos")
                nc.scalar.activation(
                    out=loga[:, t : t + 1], in_=lam_pos[:],
                    func=mybir.ActivationFunctionType.Ln, bias=epsc[:])

                # negative-pair temporals + intensity
                tdn = small.tile([P, 1], F32, tag="tdn")
                nc.vector.tensor_tensor(out=tdn[:], in0=ctt[:], in1=ltn[:], op=SUB)
                ltn1 = small.tile([P, 1], F32, tag="ltn1")
                nc.vector.tensor_scalar(
                    out=ltn1[:], in0=ltn[:], scalar1=1.0, scalar2=None, op0=ADD)
                recn = small.tile([P, 1], F32, tag="recn")
                nc.vector.reciprocal(out=recn[:], in_=ltn1[:])
                tneg = small.tile([P, 1], F32, tag="tneg")
                nc.vector.tensor_tensor(out=tneg[:], in0=tdn[:], in1=recn[:], op=MUL)
                lam_neg = intensity(d_neg[:], tneg[:], "neg")

                # survival grid: R[:, s] = sum_d uv_s * sel2 (0.5 in sel)
                # one in-place multiply over the full tile + one 3D reduce
                uv3 = uvt[:].rearrange("p (s d) -> p s d", s=S)
                sel_bS = sel2[:].rearrange("p (a d) -> p a d", a=1).to_broadcast(
                    [P, S, 2 * D])
                nc.vector.tensor_tensor(out=uv3, in0=uv3, in1=sel_bS, op=MUL)
                R = small.tile([P, S], F32, tag="R")
                nc.vector.tensor_reduce(
                    out=R[:], in_=uv3, axis=mybir.AxisListType.X, op=ADD)

                # G = R + (b + alpha*tpos/19 * s); clip/softplus/psi
                slope = small.tile([P, 1], F32, tag="slope")
                nc.vector.tensor_scalar(
                    out=slope[:], in0=tpos[:], scalar1=alpha_sel[:],
                    scalar2=1.0 / (S - 1), op0=MUL, op1=MUL)
                C = small.tile([P, S], F32, tag="C")
                nc.vector.tensor_scalar(
                    out=C[:], in0=iota_f, scalar1=slope[:],
                    scalar2=b_sel[:], op0=MUL, op1=ADD)
                G = small.tile([P, S], F32, tag="G")
                nc.vector.tensor_tensor(out=G[:], in0=R[:], in1=C[:], op=ADD)
                nc.vector.tensor_scalar(
                    out=G[:], in0=G[:], scalar1=inv_psi[:], scalar2=-75.0,
                    op0=MUL, op1=MAX)
                nc.vector.tensor_scalar(
                    out=G[:], in0=G[:], scalar1=75.0, scalar2=None, op0=MIN)
                # softplus over the grid; only the row-sum is needed:
                # sum_s relu(G_s) + sum_s ln(1 + exp(-|G_s|))
                GA = small.tile([P, S], F32, tag="GA")
                nc.scalar.activation(
                    out=GA[:], in_=G[:],
                    func=mybir.ActivationFunctionType.Abs)
                EX = small.tile([P, S], F32, tag="EX")
                nc.scalar.activation(
                    out=EX[:], in_=GA[:],
                    func=mybir.ActivationFunctionType.Exp, scale=-1.0)
                LN1 = small.tile([P, S], F32, tag="LN1")
                sum_l = small.tile([P, 1], F32, tag="sum_l")
                nc.scalar.activation(
                    out=LN1[:], in_=EX[:],
                    func=mybir.ActivationFunctionType.Ln, bias=1.0,
                    accum_out=sum_l[:])
                RL = small.tile([P, S], F32, tag="RL")
                sum_r = small.tile([P, 1], F32, tag="sum_r")
                nc.scalar.activation(
                    out=RL[:], in_=G[:],
                    func=mybir.ActivationFunctionType.Relu,
                    accum_out=sum_r[:])
                ssum = small.tile([P, 1], F32, tag="ssum")
                nc.vector.tensor_tensor(
                    out=ssum[:], in0=sum_l[:], in1=sum_r[:], op=ADD)

                tdstep = small.tile([P, 1], F32, tag="tdstep")
                nc.vector.tensor_scalar(
                    out=tdstep[:], in0=td[:], scalar1=1.0 / (S - 1),
                    scalar2=None, op0=MUL)
                nc.vector.tensor_scalar(
                    out=intega[:, t : t + 1], in0=ssum[:],
                    scalar1=psi_sel[:], scalar2=tdstep[:], op0=MUL, op1=MUL)

                surv = small.tile([P, 1], F32, tag="surv")
                nc.scalar.activation(
                    out=surv[:], in_=intega[:, t : t + 1],
                    func=mybir.ActivationFunctionType.Exp, scale=-1.0)
                nc.vector.tensor_tensor(
                    out=condp[:, t : t + 1], in0=lam_pos[:], in1=surv[:], op=MUL)
                nc.vector.tensor_tensor(
                    out=condn[:, t : t + 1], in0=lam_neg[:], in1=surv[:], op=MUL)

                if debug and t == 0:
                    for col, ap in [
                        (0, d_pos[:]), (1, d_neg[:]), (2, tpos[:]),
                        (3, b_sel[:]), (4, psi_sel[:]), (5, inv_psi[:]),
                        (6, etf[:]), (7, ltp[:]), (48, lam_pos[:]),
                        (49, intega[:, 0:1]), (50, ssum[:]),
                        (51, sum_l[:]), (52, sum_r[:]), (53, lam_neg[:]),
                        (54, td[:]), (55, surv[:]),
                    ]:
                        nc.sync.dma_start(out=out_dbg[:, col : col + 1], in_=ap)
                    nc.sync.dma_start(out=out_dbg[:, 8 : 8 + S], in_=R[:])
                    nc.sync.dma_start(out=out_dbg[:, 28 : 28 + S], in_=G[:])

            # ---- epilogue: partition-reduce losses, store outputs ------
            nc.sync.dma_start(out=out_cond[:, 0:T], in_=condp[:])
            nc.sync.dma_start(out=out_cond[:, T : 2 * T], in_=condn[:])

            ps = psump.tile([1, 2 * T], F32, tag="ps")
            nc.tensor.matmul(out=ps[:, 0:T], lhsT=ones[:], rhs=loga[:],
                             start=True, stop=True)
            nc.tensor.matmul(out=ps[:, T : 2 * T], lhsT=ones[:], rhs=intega[:],
                             start=True, stop=True)
            red = persist.tile([1, 2 * T], F32, tag="red")
            nc.scalar.copy(out=red[:], in_=ps[:])
            lvals = persist.tile([1, 2], F32, tag="lvals")
            nc.vector.tensor_reduce(
                out=lvals[:, 0:1], in_=red[:, 0:T],
                axis=mybir.AxisListType.X, op=mybir.AluOpType.add)
            nc.vector.tensor_reduce(
                out=lvals[:, 1:2], in_=red[:, T : 2 * T],
                axis=mybir.AxisListType.X, op=mybir.AluOpType.add)
            nc.vector.tensor_scalar(
                out=lvals[:, 0:1], in0=lvals[:, 0:1], scalar1=-1.0 / E,
                scalar2=None, op0=mybir.AluOpType.mult)
            nc.vector.tensor_scalar(
                out=lvals[:, 1:2], in0=lvals[:, 1:2], scalar1=1.0 / E,
                scalar2=None, op0=mybir.AluOpType.mult)
            nc.sync.dma_start(out=out_loss[:], in_=lvals[:])

    nc.finalize()
    return nc


def _get_nc():
    global _CACHED_NC
    if _CACHED_NC is None:
        _CACHED_NC = _build_nc()
    return _CACHED_NC


def _make_in_maps(inputs):
    emb = np.asarray(inputs["all_embeddings"], dtype=np.float32)
    u = np.asarray(inputs["u_non_embeddings"], dtype=np.float32)
    v = np.asarray(inputs["v_non_embeddings"], dtype=np.float32)
    lu = np.asarray(inputs["last_update"], dtype=np.float32)
    ct = np.asarray(inputs["cur_time"], dtype=np.float32)
    W = np.asarray(inputs["W"], dtype=np.float32)
    b = np.asarray(inputs["b"], dtype=np.float32)
    psi = np.asarray(inputs["psi"], dtype=np.float32)
    alpha = np.asarray(inputs["alpha"], dtype=np.float32)
    assoc = np.asarray(inputs["assoc"])
    src = np.asarray(inputs["src"])
    pos_dst = np.asarray(inputs["pos_dst"])
    neg_dst = np.asarray(inputs["neg_dst"])
    et = np.asarray(inputs["et"])

    # merged gather table: [emb | last_update]
    tab = np.ascontiguousarray(
        np.concatenate([emb, lu[:, None]], axis=1), dtype=np.float32)

    # index plumbing for the on-device gathers (assoc composition)
    idx_all = np.stack(
        [assoc[src], assoc[pos_dst], assoc[neg_dst]], axis=1
    ).astype(np.int32)
    et_i = (et > 0).astype(np.int32)[:, None]
    ct_c = np.ascontiguousarray(ct[:, None], dtype=np.float32)

    # replicated broadcast params
    wb = np.ascontiguousarray(
        np.broadcast_to(W.reshape(1, 4 * D), (P, 4 * D)), dtype=np.float32)
    parvec = np.concatenate([
        np.array([b[0], b[1], psi[0], psi[1], alpha[0], alpha[1], 0.0, 0.0],
                 dtype=np.float32),
        np.arange(S, dtype=np.float32),   # survival-step index constants
    ])
    pars = np.ascontiguousarray(np.broadcast_to(parvec, (P, 8 + S)))

    u3 = u.reshape(S, E, D)
    v3 = v.reshape(S, E, D)

    in_maps = []
    for c in range(NCORES):
        sl = slice(c * EC, (c + 1) * EC)
        # [EC, S, 2D]: u in cols 0:D, v in cols D:2D per step, contiguous
        uvc = np.concatenate(
            [u3[:, sl, :].transpose(1, 0, 2), v3[:, sl, :].transpose(1, 0, 2)],
            axis=2,
        ).reshape(EC, S * 2 * D)
        in_maps.append({
            "uv": np.ascontiguousarray(uvc),
            "tab": tab,
            "idx": np.ascontiguousarray(idx_all[sl]),
            "etd": np.ascontiguousarray(et_i[sl]),
            "ctd": ct_c[sl],
            "wb": wb,
            "pars": pars,
        })
    return in_maps


def _unshard(results):
    loss_l = np.float32(0.0)
    loss_s = np.float32(0.0)
    cond_pos = np.empty(E, dtype=np.float32)
    cond_neg = np.empty(E, dtype=np.float32)
    for c, r in enumerate(results):
        loss_l += r["out_loss"][0, 0]
        loss_s += r["out_loss"][0, 1]
        oc = r["out_cond"]  # [P, 2T]
        # event e_local = t*P + p  ->  oc[p, t]
        cond_pos[c * EC : (c + 1) * EC] = oc[:, 0:T].T.reshape(EC)
        cond_neg[c * EC : (c + 1) * EC] = oc[:, T : 2 * T].T.reshape(EC)
    return (np.float32(loss_l), np.float32(loss_s), cond_pos, cond_neg)


def kernel(**inputs):
    nc = _get_nc()
    in_maps = _make_in_maps(inputs)
    res = bass_utils.run_bass_kernel_spmd(nc, in_maps, list(range(NCORES)))
    return _unshard(res.results)


# revision 23
# speedup vs baseline: 683.0499x; 683.0499x over previous
"""Trainium2 Bass kernel for the DecoderTP temporal-Hawkes decoder.

Full-input contract: kernel(**inputs) takes the unsharded numpy inputs and
returns the full output tuple (loss_lambda/E, loss_surv/E, cond_pos, cond_neg).

Sharding: data-parallel over the event dim E (8 cores x 1024 events).
 - u/v survival embeddings are reordered host-side to per-core contiguous
   [1024, S*2*D] blocks (pure data movement).
 - all_embeddings and last_update are merged into one [N, D+1] table,
   replicated to every core; rows are gathered on-device via indirect DMA.
 - The two scalar losses are produced as per-core partials (already scaled
   by -1/E resp. 1/E) and summed host-side (the all-reduce).

Math used on device (exact algebraic rewrite of the reference):
 g_uv + g_vu = (zu+zv) @ (Wu+Wv).T  ==>  per-row fused multiply-reduce
 against per-event selected weights w_sel = wsum0 + et*(wsum1-wsum0).
"""

import numpy as np

from concourse import bass, mybir, tile
from concourse import bacc
from concourse import bass_utils

# Problem constants (hardcoded per harness contract).
E = 8192
D = 256
S = 20
N = 100000
EPS = 1e-7
NCORES = 8
EC = E // NCORES          # events per core = 1024
P = 128                   # partitions
T = EC // P               # event tiles per core = 8
ROW = D + 1               # merged table row: embedding + last_update
F32 = mybir.dt.float32
I32 = mybir.dt.int32

_CACHED_NC = None


def _build_nc(debug=False, passes=1):
    # Bacc (not raw Bass): its finalize() runs generate_event_semaphores(),
    # which legalizes multi-semaphore waits (HW allows one wait/instruction),
    # and codegen_inst_isa_subclasses() for the fused TTR instructions.
    nc = bacc.Bacc()

    uv = nc.declare_dram_parameter("uv", [EC, S * 2 * D], F32, isOutput=False)
    tab = nc.declare_dram_parameter("tab", [N, ROW], F32, isOutput=False)
    idx = nc.declare_dram_parameter("idx", [EC, 3], I32, isOutput=False)
    etd = nc.declare_dram_parameter("etd", [EC, 1], I32, isOutput=False)
    ctd = nc.declare_dram_parameter("ctd", [EC, 1], F32, isOutput=False)
    wb = nc.declare_dram_parameter("wb", [P, 4 * D], F32, isOutput=False)
    pars = nc.declare_dram_parameter("pars", [P, 8 + S], F32, isOutput=False)
    out_cond = nc.declare_dram_parameter("out_cond", [P, 2 * T], F32, isOutput=True)
    out_loss = nc.declare_dram_parameter("out_loss", [1, 2], F32, isOutput=True)
    if debug:
        out_dbg = nc.declare_dram_parameter("out_dbg", [P, 64], F32, isOutput=True)

    with tile.TileContext(nc) as tc:
        with (
            tc.tile_pool(name="big", bufs=2) as big,
            tc.tile_pool(name="gath", bufs=2) as gathp,
            tc.tile_pool(name="work", bufs=2) as work,
            tc.tile_pool(name="small", bufs=3) as small,
            tc.tile_pool(name="persist", bufs=1) as persist,
            tc.tile_pool(name="psum", bufs=1, space="PSUM") as psump,
        ):
            # ---- one-time prologue -------------------------------------
            wtile = persist.tile([P, 4 * D], F32, tag="wtile")
            nc.sync.dma_start(out=wtile[:], in_=wb[:])
            # wsum[:, 0:D] = Wu0+Wv0 ; wsum[:, D:2D] = Wu1+Wv1 (bcast rows)
            wsum = persist.tile([P, 2 * D], F32, tag="wsum")
            nc.vector.tensor_tensor(
                out=wsum[:, 0:D], in0=wtile[:, 0:D], in1=wtile[:, D : 2 * D],
                op=mybir.AluOpType.add,
            )
            nc.vector.tensor_tensor(
                out=wsum[:, D : 2 * D], in0=wtile[:, 2 * D : 3 * D],
                in1=wtile[:, 3 * D : 4 * D], op=mybir.AluOpType.add,
            )
            # fold the 0.5 of G = 0.5*(zu+zv)@(Wu+Wv).T into the weights
            nc.vector.tensor_scalar(
                out=wsum[:], in0=wsum[:], scalar1=0.5, scalar2=None,
                op0=mybir.AluOpType.mult)
            wdiff = persist.tile([P, D], F32, tag="wdiff")
            nc.vector.tensor_tensor(
                out=wdiff[:], in0=wsum[:, D : 2 * D], in1=wsum[:, 0:D],
                op=mybir.AluOpType.subtract,
            )

            parst = persist.tile([P, 8 + S], F32, tag="parst")
            nc.sync.dma_start(out=parst[:], in_=pars[:])
            # pdiff cols: 0 = b1-b0, 1 = psi1-psi0, 2 = alpha1-alpha0
            pdiff = persist.tile([P, 3], F32, tag="pdiff")
            for k in range(3):
                nc.vector.tensor_tensor(
                    out=pdiff[:, k : k + 1],
                    in0=parst[:, 2 * k + 1 : 2 * k + 2],
                    in1=parst[:, 2 * k : 2 * k + 1],
                    op=mybir.AluOpType.subtract,
                )
            p0e = persist.tile([P, 1], F32, tag="p0e")  # psi0 + EPS
            nc.vector.tensor_scalar(
                out=p0e[:], in0=parst[:, 2:3], scalar1=EPS, scalar2=None,
                op0=mybir.AluOpType.add,
            )

            iota_f = parst[:, 8 : 8 + S]

            ones = persist.tile([P, 1], F32, tag="ones")
            nc.vector.memset(ones[:], 1.0)
            epsc = persist.tile([P, 1], F32, tag="epsc")
            nc.vector.memset(epsc[:], EPS)

            # per-event accumulators (col t = event tile t)
            loga = persist.tile([P, T], F32, tag="loga")
            intega = persist.tile([P, T], F32, tag="intega")
            condp = persist.tile([P, T], F32, tag="condp")
            condn = persist.tile([P, T], F32, tag="condn")

            ADD = mybir.AluOpType.add
            SUB = mybir.AluOpType.subtract
            MUL = mybir.AluOpType.mult
            MAX = mybir.AluOpType.max
            MIN = mybir.AluOpType.min

            # ---- main loop over event tiles ----------------------------
            # passes>1 repeats the whole computation (for slope-timing).
            for t_rep in range(passes * T):
                t = t_rep % T
                r0 = t * P
                uvt = big.tile([P, S * 2 * D], F32, tag="uvt")
                nc.sync.dma_start(out=uvt[:], in_=uv[r0 : r0 + P, :])

                idxt = small.tile([P, 3], I32, tag="idxt")
                nc.sync.dma_start(out=idxt[:], in_=idx[r0 : r0 + P, :])
                ett = small.tile([P, 1], I32, tag="ett")
                nc.sync.dma_start(out=ett[:], in_=etd[r0 : r0 + P, :])
                ctt = small.tile([P, 1], F32, tag="ctt")
                nc.sync.dma_start(out=ctt[:], in_=ctd[r0 : r0 + P, :])

                gth = gathp.tile([P, 3 * ROW], F32, tag="gth")
                for j in range(3):
                    nc.gpsimd.indirect_dma_start(
                        out=gth[:, j * ROW : (j + 1) * ROW],
                        out_offset=None,
                        in_=tab[:],
                        in_offset=bass.IndirectOffsetOnAxis(
                            ap=idxt[:, j : j + 1], axis=0
                        ),
                    )

                etf = small.tile([P, 1], F32, tag="etf")
                nc.vector.tensor_copy(out=etf[:], in_=ett[:])

                # per-event selected params
                b_sel = small.tile([P, 1], F32, tag="b_sel")
                nc.vector.tensor_scalar(
                    out=b_sel[:], in0=etf[:], scalar1=pdiff[:, 0:1],
                    scalar2=parst[:, 0:1], op0=MUL, op1=ADD)
                psi_sel = small.tile([P, 1], F32, tag="psi_sel")
                nc.vector.tensor_scalar(
                    out=psi_sel[:], in0=etf[:], scalar1=pdiff[:, 1:2],
                    scalar2=parst[:, 2:3], op0=MUL, op1=ADD)
                psi_eps = small.tile([P, 1], F32, tag="psi_eps")
                nc.vector.tensor_scalar(
                    out=psi_eps[:], in0=etf[:], scalar1=pdiff[:, 1:2],
                    scalar2=p0e[:], op0=MUL, op1=ADD)
                inv_psi = small.tile([P, 1], F32, tag="inv_psi")
                nc.vector.reciprocal(out=inv_psi[:], in_=psi_eps[:])
                alpha_sel = small.tile([P, 1], F32, tag="alpha_sel")
                nc.vector.tensor_scalar(
                    out=alpha_sel[:], in0=etf[:], scalar1=pdiff[:, 2:3],
                    scalar2=parst[:, 4:5], op0=MUL, op1=ADD)

                # selected weight row: wsum0 + et*(wsum1-wsum0), duplicated
                # into both halves of sel2 so uv-step rows multiply directly
                sel2 = work.tile([P, 2 * D], F32, tag="sel2")
                nc.vector.tensor_scalar(
                    out=sel2[:, 0:D], in0=wdiff[:], scalar1=etf[:],
                    scalar2=None, op0=MUL)
                nc.vector.tensor_tensor(
                    out=sel2[:, 0:D], in0=sel2[:, 0:D], in1=wsum[:, 0:D],
                    op=ADD)
                nc.vector.tensor_copy(out=sel2[:, D : 2 * D], in_=sel2[:, 0:D])
                sel = sel2[:, 0:D]

                lu_src = gth[:, D : D + 1]
                lu_pos = gth[:, ROW + D : ROW + D + 1]
                lu_neg = gth[:, 2 * ROW + D : 2 * ROW + D + 1]

                # Temporals first: these standard DVE ops are the first
                # consumers of the gathered tile + the uv stream, so they
                # absorb the DMA semaphore waits (the fused TTR ISA insts
                # below have very few sync-wait slots).
                ltp = small.tile([P, 1], F32, tag="ltp")
                nc.vector.tensor_tensor(out=ltp[:], in0=lu_src, in1=lu_pos, op=MAX)
                ltn = small.tile([P, 1], F32, tag="ltn")
                nc.vector.tensor_tensor(out=ltn[:], in0=lu_src, in1=lu_neg, op=MAX)
                touch = small.tile([P, 1], F32, tag="touch")
                nc.vector.tensor_copy(out=touch[:], in_=uvt[:, 0:1])

                td = small.tile([P, 1], F32, tag="td")
                nc.vector.tensor_tensor(out=td[:], in0=ctt[:], in1=ltp[:], op=SUB)
                lt1 = small.tile([P, 1], F32, tag="lt1")
                nc.vector.tensor_scalar(
                    out=lt1[:], in0=ltp[:], scalar1=1.0, scalar2=None, op0=ADD)
                rec = small.tile([P, 1], F32, tag="rec")
                nc.vector.reciprocal(out=rec[:], in_=lt1[:])
                tpos = small.tile([P, 1], F32, tag="tpos")
                nc.vector.tensor_tensor(out=tpos[:], in0=td[:], in1=rec[:], op=MUL)

                # dot products for pos/neg pairs: 0.5 factor is in sel.
                # z_src*sel is shared between the pos and neg dots.
                scr = work.tile([P, 2 * D], F32, tag="scr")
                scr3 = scr[:].rearrange("p (a b) -> p a b", a=2)
                nc.vector.tensor_tensor(
                    out=scr[:, 0:D], in0=gth[:, 0:D], in1=sel, op=MUL)
                nc.vector.tensor_tensor(
                    out=scr[:, D : 2 * D], in0=gth[:, ROW : ROW + D],
                    in1=sel, op=MUL)
                d_pos = small.tile([P, 1], F32, tag="d_pos")
                nc.vector.tensor_reduce(
                    out=d_pos[:], in_=scr3, axis=mybir.AxisListType.XY,
                    op=ADD)
                nc.vector.tensor_tensor(
                    out=scr[:, D : 2 * D], in0=gth[:, 2 * ROW : 2 * ROW + D],
                    in1=sel, op=MUL)
                d_neg = small.tile([P, 1], F32, tag="d_neg")
                nc.vector.tensor_reduce(
                    out=d_neg[:], in_=scr3, axis=mybir.AxisListType.XY,
                    op=ADD)

                def intensity(dot_ap, temporal_ap, tag):
                    # psi * softplus(clip((dot + b + alpha*temporal)/psi_eps))
                    g = small.tile([P, 1], F32, tag=tag + "_g")
                    nc.vector.tensor_scalar(
                        out=g[:], in0=temporal_ap, scalar1=alpha_sel[:],
                        scalar2=b_sel[:], op0=MUL, op1=ADD)
                    nc.vector.tensor_tensor(out=g[:], in0=g[:], in1=dot_ap, op=ADD)
                    nc.vector.tensor_scalar(
                        out=g[:], in0=g[:], scalar1=inv_psi[:], scalar2=-75.0,
                        op0=MUL, op1=MAX)
                    nc.vector.tensor_scalar(
                        out=g[:], in0=g[:], scalar1=75.0, scalar2=None, op0=MIN)
                    # stable softplus: relu(g) + ln(1 + exp(-|g|))
                    ga = small.tile([P, 1], F32, tag=tag + "_ga")
                    nc.scalar.activation(
                        out=ga[:], in_=g[:],
                        func=mybir.ActivationFunctionType.Abs)
                    ex = small.tile([P, 1], F32, tag=tag + "_ex")
                    nc.scalar.activation(
                        out=ex[:], in_=ga[:],
                        func=mybir.ActivationFunctionType.Exp, scale=-1.0)
                    ln1 = small.tile([P, 1], F32, tag=tag + "_ln1")
                    nc.scalar.activation(
                        out=ln1[:], in_=ex[:],
                        func=mybir.ActivationFunctionType.Ln, bias=1.0)
                    rl = small.tile([P, 1], F32, tag=tag + "_rl")
                    nc.scalar.activation(
                        out=rl[:], in_=g[:],
                        func=mybir.ActivationFunctionType.Relu)
                    sp = small.tile([P, 1], F32, tag=tag + "_sp")
                    nc.vector.tensor_tensor(
                        out=sp[:], in0=rl[:], in1=ln1[:], op=ADD)
                    lam = small.tile([P, 1], F32, tag=tag + "_lam")
                    nc.vector.tensor_scalar(
                        out=lam[:], in0=sp[:], scalar1=psi_sel[:],
                        scalar2=None, op0=MUL)
                    return lam

                lam_pos = intensity(d_pos[:], tpos[:], "pos")
                nc.scalar.activation(
                    out=loga[:, t : t + 1], in_=lam_pos[:],
                    func=mybir.ActivationFunctionType.Ln, bias=epsc[:])

                # negative-pair temporals + intensity
                tdn = small.tile([P, 1], F32, tag="tdn")
                nc.vector.tensor_tensor(out=tdn[:], in0=ctt[:], in1=ltn[:], op=SUB)
                ltn1 = small.tile([P, 1], F32, tag="ltn1")
                nc.vector.tensor_scalar(
                    out=ltn1[:], in0=ltn[:], scalar1=1.0, scalar2=None, op0=ADD)
                recn = small.tile([P, 1], F32, tag="recn")
                nc.vector.reciprocal(out=recn[:], in_=ltn1[:])
                tneg = small.tile([P, 1], F32, tag="tneg")
                nc.vector.tensor_tensor(out=tneg[:], in0=tdn[:], in1=recn[:], op=MUL)
                lam_neg = intensity(d_neg[:], tneg[:], "neg")

                # survival grid: R[:, s] = sum_d uv_s * sel2 (0.5 in sel)
                # one in-place multiply over the full tile + one 3D reduce
                uv3 = uvt[:].rearrange("p (s d) -> p s d", s=S)
                sel_bS = sel2[:].rearrange("p (a d) -> p a d", a=1).to_broadcast(
                    [P, S, 2 * D])
                nc.vector.tensor_tensor(out=uv3, in0=uv3, in1=sel_bS, op=MUL)
                R = small.tile([P, S], F32, tag="R")
                nc.vector.tensor_reduce(
                    out=R[:], in_=uv3, axis=mybir.AxisListType.X, op=ADD)

                # G = R + (b + alpha*tpos/19 * s); clip/softplus/psi
                slope = small.tile([P, 1], F32, tag="slope")
                nc.vector.tensor_scalar(
                    out=slope[:], in0=tpos[:], scalar1=alpha_sel[:],
                    scalar2=1.0 / (S - 1), op0=MUL, op1=MUL)
                C = small.tile([P, S], F32, tag="C")
                nc.vector.tensor_scalar(
                    out=C[:], in0=iota_f, scalar1=slope[:],
                    scalar2=b_sel[:], op0=MUL, op1=ADD)
                G = small.tile([P, S], F32, tag="G")
                nc.vector.tensor_tensor(out=G[:], in0=R[:], in1=C[:], op=ADD)
                nc.vector.tensor_scalar(
                    out=G[:], in0=G[:], scalar1=inv_psi[:], scalar2=-75.0,
                    op0=MUL, op1=MAX)
                nc.vector.tensor_scalar(
                    out=G[:], in0=G[:], scalar1=75.0, scalar2=None, op0=MIN)
                # softplus over the grid; only the row-sum is needed:
                # sum_s relu(G_s) + sum_s ln(1 + exp(-|G_s|))
                GA = small.tile([P, S], F32, tag="GA")
                nc.scalar.activation(
                    out=GA[:], in_=G[:],
                    func=mybir.ActivationFunctionType.Abs)
                EX = small.tile([P, S], F32, tag="EX")
                nc.scalar.activation(
                    out=EX[:], in_=GA[:],
                    func=mybir.ActivationFunctionType.Exp, scale=-1.0)
                LN1 = small.tile([P, S], F32, tag="LN1")
                sum_l = small.tile([P, 1], F32, tag="sum_l")
                nc.scalar.activation(
                    out=LN1[:], in_=EX[:],
                    func=mybir.ActivationFunctionType.Ln, bias=1.0,
                    accum_out=sum_l[:])
                RL = small.tile([P, S], F32, tag="RL")
                sum_r = small.tile([P, 1], F32, tag="sum_r")
                nc.scalar.activation(
                    out=RL[:], in_=G[:],
                    func=mybir.ActivationFunctionType.Relu,
                    accum_out=sum_r[:])
                ssum = small.tile([P, 1], F32, tag="ssum")
                nc.vector.tensor_tensor(
                    out=ssum[:], in0=sum_l[:], in1=sum_r[:], op=ADD)

                tdstep = small.tile([P, 1], F32, tag="tdstep")
                nc.vector.tensor_scalar(
                    out=tdstep[:], in0=td[:], scalar1=1.0 / (S - 1),
                    scalar2=None, op0=MUL)
                nc.vector.tensor_scalar(
                    out=intega[:, t : t + 1], in0=ssum[:],
                    scalar1=psi_sel[:], scalar2=tdstep[:], op0=MUL, op1=MUL)

                surv = small.tile([P, 1], F32, tag="surv")
                nc.scalar.activation(
                    out=surv[:], in_=intega[:, t : t + 1],
                    func=mybir.ActivationFunctionType.Exp, scale=-1.0)
                nc.vector.tensor_tensor(
                    out=condp[:, t : t + 1], in0=lam_pos[:], in1=surv[:], op=MUL)
                nc.vector.tensor_tensor(
                    out=condn[:, t : t + 1], in0=lam_neg[:], in1=surv[:], op=MUL)

                if debug and t == 0:
                    for col, ap in [
                        (0, d_pos[:]), (1, d_neg[:]), (2, tpos[:]),
                        (3, b_sel[:]), (4, psi_sel[:]), (5, inv_psi[:]),
                        (6, etf[:]), (7, ltp[:]), (48, lam_pos[:]),
                        (49, intega[:, 0:1]), (50, ssum[:]),
                        (51, sum_l[:]), (52, sum_r[:]), (53, lam_neg[:]),
                        (54, td[:]), (55, surv[:]),
                    ]:
                        nc.sync.dma_start(out=out_dbg[:, col : col + 1], in_=ap)
                    nc.sync.dma_start(out=out_dbg[:, 8 : 8 + S], in_=R[:])
                    nc.sync.dma_start(out=out_dbg[:, 28 : 28 + S], in_=G[:])

            # ---- epilogue: partition-reduce losses, store outputs ------
            nc.sync.dma_start(out=out_cond[:, 0:T], in_=condp[:])
            nc.sync.dma_start(out=out_cond[:, T : 2 * T], in_=condn[:])

            ps = psump.tile([1, 2 * T], F32, tag="ps")
            nc.tensor.matmul(out=ps[:, 0:T], lhsT=ones[:], rhs=loga[:],
                             start=True, stop=True)
            nc.tensor.matmul(out=ps[:, T : 2 * T], lhsT=ones[:], rhs=intega[:],
                             start=True, stop=True)
            red = persist.tile([1, 2 * T], F32, tag="red")
            nc.scalar.copy(out=red[:], in_=ps[:])
            lvals = persist.tile([1, 2], F32, tag="lvals")
            nc.vector.tensor_reduce(
                out=lvals[:, 0:1], in_=red[:, 0:T],
                axis=mybir.AxisListType.X, op=mybir.AluOpType.add)
            nc.vector.tensor_reduce(
                out=lvals[:, 1:2], in_=red[:, T : 2 * T],
                axis=mybir.AxisListType.X, op=mybir.AluOpType.add)
            nc.vector.tensor_scalar(
                out=lvals[:, 0:1], in0=lvals[:, 0:1], scalar1=-1.0 / E,
                scalar2=None, op0=mybir.AluOpType.mult)
            nc.vector.tensor_scalar(
                out=lvals[:, 1:2], in0=lvals[:, 1:2], scalar1=1.0 / E,
                scalar2=None, op0=mybir.AluOpType.mult)
            nc.sync.dma_start(out=out_loss[:], in_=lvals[:])

    nc.finalize()
    return nc


def _get_nc():
    global _CACHED_NC
    if _CACHED_NC is None:
        _CACHED_NC = _build_nc()
    return _CACHED_NC


def _make_in_maps(inputs):
    emb = np.asarray(inputs["all_embeddings"], dtype=np.float32)
    u = np.asarray(inputs["u_non_embeddings"], dtype=np.float32)
    v = np.asarray(inputs["v_non_embeddings"], dtype=np.float32)
    lu = np.asarray(inputs["last_update"], dtype=np.float32)
    ct = np.asarray(inputs["cur_time"], dtype=np.float32)
    W = np.asarray(inputs["W"], dtype=np.float32)
    b = np.asarray(inputs["b"], dtype=np.float32)
    psi = np.asarray(inputs["psi"], dtype=np.float32)
    alpha = np.asarray(inputs["alpha"], dtype=np.float32)
    assoc = np.asarray(inputs["assoc"])
    src = np.asarray(inputs["src"])
    pos_dst = np.asarray(inputs["pos_dst"])
    neg_dst = np.asarray(inputs["neg_dst"])
    et = np.asarray(inputs["et"])

    # merged gather table: [emb | last_update]
    tab = np.ascontiguousarray(
        np.concatenate([emb, lu[:, None]], axis=1), dtype=np.float32)

    # index plumbing for the on-device gathers (assoc composition)
    idx_all = np.stack(
        [assoc[src], assoc[pos_dst], assoc[neg_dst]], axis=1
    ).astype(np.int32)
    et_i = (et > 0).astype(np.int32)[:, None]
    ct_c = np.ascontiguousarray(ct[:, None], dtype=np.float32)

    # replicated broadcast params
    wb = np.ascontiguousarray(
        np.broadcast_to(W.reshape(1, 4 * D), (P, 4 * D)), dtype=np.float32)
    parvec = np.concatenate([
        np.array([b[0], b[1], psi[0], psi[1], alpha[0], alpha[1], 0.0, 0.0],
                 dtype=np.float32),
        np.arange(S, dtype=np.float32),   # survival-step index constants
    ])
    pars = np.ascontiguousarray(np.broadcast_to(parvec, (P, 8 + S)))

    u3 = u.reshape(S, E, D)
    v3 = v.reshape(S, E, D)

    in_maps = []
    for c in range(NCORES):
        sl = slice(c * EC, (c + 1) * EC)
        # [EC, S, 2D]: u in cols 0:D, v in cols D:2D per step, contiguous
        uvc = np.concatenate(
            [u3[:, sl, :].transpose(1, 0, 2), v3[:, sl, :].transpose(1, 0, 2)],
            axis=2,
        ).reshape(EC, S * 2 * D)
        in_maps.append({
            "uv": np.ascontiguousarray(uvc),
            "tab": tab,
            "idx": np.ascontiguousarray(idx_all[sl]),
            "etd": np.ascontiguousarray(et_i[sl]),
            "ctd": ct_c[sl],
            "wb": wb,
            "pars": pars,
        })
    return in_maps


def _unshard(results):
    loss_l = np.float32(0.0)
    loss_s = np.float32(0.0)
    cond_pos = np.empty(E, dtype=np.float32)
    cond_neg = np.empty(E, dtype=np.float32)
    for c, r in enumerate(results):
        loss_l += r["out_loss"][0, 0]
        loss_s += r["out_loss"][0, 1]
        oc = r["out_cond"]  # [P, 2T]
        # event e_local = t*P + p  ->  oc[p, t]
        cond_pos[c * EC : (c + 1) * EC] = oc[:, 0:T].T.reshape(EC)
        cond_neg[c * EC : (c + 1) * EC] = oc[:, T : 2 * T].T.reshape(EC)
    return (np.float32(loss_l), np.float32(loss_s), cond_pos, cond_neg)


def kernel(**inputs):
    nc = _get_nc()
    in_maps = _make_in_maps(inputs)
    res = bass_utils.run_bass_kernel_spmd(nc, in_maps, list(range(NCORES)))
    return _unshard(res.results)
